# revision 10
# baseline (speedup 1.0000x reference)
"""MoE (AriaExperts) Trainium2 kernel — expert parallelism across 8 NeuronCores.

Strategy:
  - Host: top-2 routing + softmax over [2048, 8] logits (tiny), build the
    per-expert token batches (the "all-to-all" is realized at input
    distribution time), and the weighted scatter-add combine at the end.
  - Device (SPMD, 1 expert per core): dense GEMM chain in transposed
    activation layout so both matmuls consume the expert weights directly
    as the stationary (lhsT) operand with zero on-device transposes:
        H^T  = W1^T-tiles @ X^T      [2*INTER, C]
        actT = silu(projT) * gateT   [INTER, C]
        outT = W2-tiles   @ actT     [HIDDEN, C]
    bf16 matmuls with f32 PSUM accumulation (1 cycle/row vs 4 for f32).
  - Each core processes C = (max expert token count, padded) columns; the
    shapes are chosen at runtime from the actual routing, the graph is
    compiled per-shape and cached.
"""

import numpy as np
import ml_dtypes

import concourse.bass as bass
import concourse.bacc as bacc
import concourse.mybir as mybir
import concourse.tile as tile
from concourse.bass_utils import run_bass_kernel_spmd

NUM_TOKENS = 2048
HIDDEN = 1024
INTER = 2048
NUM_EXPERTS = 8
TOPK = 2
NCORES = 8
P = 128
KT1 = HIDDEN // P         # 8  k-tiles (FC1 contraction)
MT1 = 2 * INTER // P      # 32 m-tiles (FC1 output rows = proj+gate)
MT1H = INTER // P         # 16 proj/gate pair count
KT2 = INTER // P          # 16 k-tiles (FC2 contraction)
MT2 = HIDDEN // P         # 8  m-tiles (FC2 output rows)

BF16 = mybir.dt.bfloat16
F32 = mybir.dt.float32
np_bf16 = ml_dtypes.bfloat16

_graph_cache: dict = {}


def _build(C_pad: int, n_chunks: int, CH: int) -> bass.Bass:
    """Build the per-core Bass graph for capacity C_pad = n_chunks * CH."""
    nc = bacc.Bacc("TRN2", target_bir_lowering=False, debug=False)

    xt_d = nc.declare_dram_parameter("xt", [P, KT1, C_pad], BF16, isOutput=False)
    w1_d = nc.declare_dram_parameter("w1", [P, MT1, KT1, P], BF16, isOutput=False)
    w2_d = nc.declare_dram_parameter("w2", [P, MT2, KT2, P], BF16, isOutput=False)
    out_d = nc.declare_dram_parameter(
        "out", [MT2, P, n_chunks, CH], F32, isOutput=True
    )

    psum_bufs = max(2, 8 // (2 * n_chunks)) * 2  # 8 banks total

    with tile.TileContext(nc) as tc:
        with (
            tc.tile_pool(name="weights", bufs=1) as wpool,
            tc.tile_pool(name="xact", bufs=1) as xpool,
            tc.tile_pool(name="tmp", bufs=2) as tpool,
            tc.tile_pool(name="osb", bufs=2) as opool,
            tc.tile_pool(name="psum", bufs=psum_bufs, space="PSUM") as pspool,
        ):
            xt = xpool.tile([P, KT1, C_pad], BF16, tag="xt")
            act = xpool.tile([P, KT2, n_chunks, CH], BF16, tag="act")
            w1 = wpool.tile([P, MT1, KT1, P], BF16, tag="w1")
            w2 = wpool.tile([P, MT2, KT2, P], BF16, tag="w2")

            nc.sync.dma_start(out=xt[:], in_=xt_d[:])
            # W1 in chunks of 4 m-tiles (~1 MiB each) so FC1 starts early.
            for g in range(MT1 // 4):
                nc.sync.dma_start(
                    out=w1[:, g * 4 : (g + 1) * 4], in_=w1_d[:, g * 4 : (g + 1) * 4]
                )
            for g in range(MT2 // 4):
                nc.sync.dma_start(
                    out=w2[:, g * 4 : (g + 1) * 4], in_=w2_d[:, g * 4 : (g + 1) * 4]
                )

            # ---- FC1 (proj/gate pair per iteration) + SwiGLU ----
            for mt in range(MT1H):
                ps_p = pspool.tile([P, n_chunks, 512], F32, tag="ps", name=f"psp{mt}")
                ps_g = pspool.tile([P, n_chunks, 512], F32, tag="ps", name=f"psg{mt}")
                for ps, m in ((ps_p, mt), (ps_g, mt + MT1H)):
                    for kt in range(KT1):
                        for j in range(n_chunks):
                            nc.tensor.matmul(
                                ps[:, j, :CH],
                                w1[:, m, kt, :],
                                xt[:, kt, j * CH : (j + 1) * CH],
                                start=(kt == 0),
                                stop=(kt == KT1 - 1),
                            )
                # SwiGLU: silu on ACT, multiply on DVE. Multi-wait
                # instructions are legal here because Bacc.compile()'s
                # generate_event_semaphores() splits them (hardware allows
                # one sync-wait per compute instruction).
                tmp = tpool.tile([P, n_chunks, CH], F32, tag="tmp", name=f"tmp{mt}")
                for j in range(n_chunks):
                    nc.scalar.activation(
                        tmp[:, j], ps_p[:, j, :CH], mybir.ActivationFunctionType.Silu
                    )
                    nc.vector.tensor_mul(act[:, mt, j], tmp[:, j], ps_g[:, j, :CH])

            # ---- FC2 ----
            for m2 in range(MT2):
                ps_o = pspool.tile([P, n_chunks, 512], F32, tag="ps", name=f"pso{m2}")
                for kt2 in range(KT2):
                    for j in range(n_chunks):
                        nc.tensor.matmul(
                            ps_o[:, j, :CH],
                            w2[:, m2, kt2, :],
                            act[:, kt2, j, :],
                            start=(kt2 == 0),
                            stop=(kt2 == KT2 - 1),
                        )
                o_sb = opool.tile([P, n_chunks, CH], F32, tag="o", name=f"osb{m2}")
                for j in range(n_chunks):
                    nc.scalar.copy(o_sb[:, j], ps_o[:, j, :CH])
                nc.sync.dma_start(out=out_d[m2], in_=o_sb[:])

    nc.compile()
    return nc


def _get_graph(C_pad: int, n_chunks: int, CH: int) -> bass.Bass:
    key = (C_pad, n_chunks, CH)
    if key not in _graph_cache:
        _graph_cache[key] = _build(C_pad, n_chunks, CH)
    return _graph_cache[key]


def _route(router_logits: np.ndarray):
    """Top-2 + softmax, exactly matching jax.lax.top_k tie-breaking."""
    idx = np.argsort(-router_logits, axis=-1, kind="stable")[:, :TOPK]
    tl = np.take_along_axis(router_logits, idx, axis=-1)
    ex = np.exp(tl - tl.max(-1, keepdims=True))
    sc = (ex / ex.sum(-1, keepdims=True)).astype(np.float32)
    return idx, sc


def run(hidden_states, router_logits, w1, w2, trace=False, trace_kwargs=None):
    hs = np.asarray(hidden_states, dtype=np.float32)
    rl = np.asarray(router_logits, dtype=np.float32)
    w1 = np.asarray(w1, dtype=np.float32)
    w2 = np.asarray(w2, dtype=np.float32)
    N, D = hs.shape

    idx, sc = _route(rl)

    tok_lists = []
    for e in range(NUM_EXPERTS):
        toks, slots = np.nonzero(idx == e)
        tok_lists.append((toks, slots))
    cmax = max(len(t) for t, _ in tok_lists)

    n_chunks = max(1, -(-cmax // 512))
    CH = -(-cmax // (n_chunks * 16)) * 16  # chunk width, multiple of 16
    C_pad = CH * n_chunks

    in_maps = []
    for e in range(NUM_EXPERTS):
        toks, _ = tok_lists[e]
        x = np.zeros((C_pad, D), np.float32)
        x[: len(toks)] = hs[toks]
        xt = x.T.reshape(KT1, P, C_pad).transpose(1, 0, 2).astype(np_bf16)
        w1e = w1[e].reshape(KT1, P, MT1, P).transpose(1, 2, 0, 3).astype(np_bf16)
        w2e = w2[e].reshape(KT2, P, MT2, P).transpose(1, 2, 0, 3).astype(np_bf16)
        in_maps.append({"xt": xt, "w1": w1e, "w2": w2e})

    nc = _get_graph(C_pad, n_chunks, CH)
    res = run_bass_kernel_spmd(
        nc,
        in_maps,
        core_ids=list(range(NCORES)),
        trace=trace,
        **(trace_kwargs or {}),
    )

    out = np.zeros((N, D), np.float32)
    for e in range(NUM_EXPERTS):
        toks, slots = tok_lists[e]
        oT = np.asarray(res.results[e]["out"], np.float32).reshape(HIDDEN, C_pad)
        out[toks] += sc[toks, slots][:, None] * oT[:, : len(toks)].T
    return out, res


def kernel(hidden_states, router_logits, w1, w2):
    out, _ = run(hidden_states, router_logits, w1, w2)
    return out


# revision 13
# speedup vs baseline: 1.0856x; 1.0856x over previous
"""MoE (AriaExperts) Trainium2 kernel — expert parallelism across 8 NeuronCores.

Strategy:
  - Host: top-2 routing + softmax over [2048, 8] logits (tiny), build the
    per-expert token batches (the "all-to-all" is realized at input
    distribution time), and the weighted scatter-add combine at the end.
  - Device (SPMD, 1 expert per core): dense GEMM chain in transposed
    activation layout so both matmuls consume the expert weights directly
    as the stationary (lhsT) operand with zero on-device transposes:
        H^T  = W1^T-tiles @ X^T      [2*INTER, C]
        actT = silu(projT) * gateT   [INTER, C]
        outT = W2-tiles   @ actT     [HIDDEN, C]
    bf16 matmuls with f32 PSUM accumulation (1 cycle/row vs 4 for f32).
  - Each core processes C = (max expert token count, padded) columns; the
    shapes are chosen at runtime from the actual routing, the graph is
    compiled per-shape and cached.
"""

import numpy as np
import ml_dtypes

import concourse.bass as bass
import concourse.bacc as bacc
import concourse.mybir as mybir
import concourse.tile as tile
from concourse.bass_utils import run_bass_kernel_spmd

NUM_TOKENS = 2048
HIDDEN = 1024
INTER = 2048
NUM_EXPERTS = 8
TOPK = 2
NCORES = 8
P = 128
KT1 = HIDDEN // P         # 8  k-tiles (FC1 contraction)
MT1 = 2 * INTER // P      # 32 m-tiles (FC1 output rows = proj+gate)
MT1H = INTER // P         # 16 proj/gate pair count
KT2 = INTER // P          # 16 k-tiles (FC2 contraction)
MT2 = HIDDEN // P         # 8  m-tiles (FC2 output rows)

BF16 = mybir.dt.bfloat16
F32 = mybir.dt.float32
np_bf16 = ml_dtypes.bfloat16

# [0, 16, 1, 17, ...] — interleave proj/gate m-tiles into adjacent pairs
_W1_ORDER = np.arange(MT1).reshape(2, MT1H).T.reshape(-1)

_graph_cache: dict = {}


def _build(C_pad: int, n_chunks: int, CH: int) -> bass.Bass:
    """Build the per-core Bass graph for capacity C_pad = n_chunks * CH."""
    nc = bacc.Bacc("TRN2", target_bir_lowering=False, debug=False)

    xt_d = nc.declare_dram_parameter("xt", [P, KT1, C_pad], BF16, isOutput=False)
    w1_d = nc.declare_dram_parameter("w1", [P, MT1, KT1, P], BF16, isOutput=False)
    w2_d = nc.declare_dram_parameter("w2", [P, MT2, KT2, P], BF16, isOutput=False)
    out_d = nc.declare_dram_parameter(
        "out", [MT2, P, n_chunks, CH], F32, isOutput=True
    )

    psum_bufs = max(2, 8 // (2 * n_chunks)) * 2  # 8 banks total

    # w1 DMA chunk sizes in proj/gate PAIRS (host layout interleaves
    # proj mt / gate mt+16 adjacently so pair mt only needs chunk ~mt/2):
    # fine-grained at the front so the first pairs start ASAP.
    w1_chunks = [1, 1, 2, 2, 2, 2, 2, 2, 2]
    assert sum(w1_chunks) == MT1H

    with tile.TileContext(nc) as tc:
        with (
            tc.tile_pool(name="weights", bufs=1) as wpool,
            tc.tile_pool(name="xact", bufs=1) as xpool,
            tc.tile_pool(name="tmp", bufs=2) as tpool,
            tc.tile_pool(name="osb", bufs=2) as opool,
            tc.tile_pool(name="psum", bufs=psum_bufs, space="PSUM") as pspool,
        ):
            xt = xpool.tile([P, KT1, C_pad], BF16, tag="xt")
            act = xpool.tile([P, KT2, n_chunks, CH], BF16, tag="act")
            w1 = wpool.tile([P, MT1H, 2, KT1, P], BF16, tag="w1")
            w2 = wpool.tile([P, MT2, KT2, P], BF16, tag="w2")
            dummy = xpool.tile([P, 640], BF16, tag="dummy")

            # PE warmup: ~20 back-to-back matmuls on a memset tile so the
            # HAM clock-gate reaches K=8/8 while input DMAs are in flight
            # (otherwise the first ~15us of real matmuls run at 1.2 GHz).
            nc.gpsimd.memset(dummy[:], 0.0)
            warm_ps = pspool.tile([P, n_chunks, 512], F32, tag="ps", name="warmps")
            for _ in range(20):
                nc.tensor.matmul(
                    warm_ps[:, 0, :], dummy[:, :128], dummy[:, 128:640],
                    start=True, stop=True,
                )

            # Input DMAs on BOTH HWDGE rings (SP + ACT) — triggers cost
            # ~650ns each and serialize per ring.
            nc.sync.dma_start(out=xt[:], in_=xt_d[:])
            pair0 = 0
            for ci, cw in enumerate(w1_chunks):
                eng = nc.scalar if ci % 2 == 0 else nc.sync
                eng.dma_start(
                    out=w1[:, pair0 : pair0 + cw],
                    in_=w1_d[:, 2 * pair0 : 2 * (pair0 + cw)],
                )
                pair0 += cw
            for g in range(MT2 // 4):
                eng = nc.scalar if g % 2 == 0 else nc.sync
                eng.dma_start(
                    out=w2[:, g * 4 : (g + 1) * 4], in_=w2_d[:, g * 4 : (g + 1) * 4]
                )

            # ---- FC1 (proj/gate pair per iteration) + SwiGLU ----
            for mt in range(MT1H):
                ps_p = pspool.tile([P, n_chunks, 512], F32, tag="ps", name=f"psp{mt}")
                ps_g = pspool.tile([P, n_chunks, 512], F32, tag="ps", name=f"psg{mt}")
                for ps, pg in ((ps_p, 0), (ps_g, 1)):
                    for kt in range(KT1):
                        for j in range(n_chunks):
                            nc.tensor.matmul(
                                ps[:, j, :CH],
                                w1[:, mt, pg, kt, :],
                                xt[:, kt, j * CH : (j + 1) * CH],
                                start=(kt == 0),
                                stop=(kt == KT1 - 1),
                            )
                # SwiGLU: silu on ACT, multiply on DVE. Multi-wait
                # instructions are legal here because Bacc.compile()'s
                # generate_event_semaphores() splits them (hardware allows
                # one sync-wait per compute instruction).
                tmp = tpool.tile([P, n_chunks, CH], F32, tag="tmp", name=f"tmp{mt}")
                for j in range(n_chunks):
                    nc.scalar.activation(
                        tmp[:, j], ps_p[:, j, :CH], mybir.ActivationFunctionType.Silu
                    )
                    nc.vector.tensor_mul(act[:, mt, j], tmp[:, j], ps_g[:, j, :CH])

            # ---- FC2 ----
            for m2 in range(MT2):
                ps_o = pspool.tile([P, n_chunks, 512], F32, tag="ps", name=f"pso{m2}")
                for kt2 in range(KT2):
                    for j in range(n_chunks):
                        nc.tensor.matmul(
                            ps_o[:, j, :CH],
                            w2[:, m2, kt2, :],
                            act[:, kt2, j, :],
                            start=(kt2 == 0),
                            stop=(kt2 == KT2 - 1),
                        )
                o_sb = opool.tile([P, n_chunks, CH], F32, tag="o", name=f"osb{m2}")
                for j in range(n_chunks):
                    nc.scalar.copy(o_sb[:, j], ps_o[:, j, :CH])
                nc.sync.dma_start(out=out_d[m2], in_=o_sb[:])

    nc.compile()
    return nc


def _get_graph(C_pad: int, n_chunks: int, CH: int) -> bass.Bass:
    key = (C_pad, n_chunks, CH)
    if key not in _graph_cache:
        _graph_cache[key] = _build(C_pad, n_chunks, CH)
    return _graph_cache[key]


def _route(router_logits: np.ndarray):
    """Top-2 + softmax, exactly matching jax.lax.top_k tie-breaking."""
    idx = np.argsort(-router_logits, axis=-1, kind="stable")[:, :TOPK]
    tl = np.take_along_axis(router_logits, idx, axis=-1)
    ex = np.exp(tl - tl.max(-1, keepdims=True))
    sc = (ex / ex.sum(-1, keepdims=True)).astype(np.float32)
    return idx, sc


def run(hidden_states, router_logits, w1, w2, trace=False, trace_kwargs=None):
    hs = np.asarray(hidden_states, dtype=np.float32)
    rl = np.asarray(router_logits, dtype=np.float32)
    w1 = np.asarray(w1, dtype=np.float32)
    w2 = np.asarray(w2, dtype=np.float32)
    N, D = hs.shape

    idx, sc = _route(rl)

    tok_lists = []
    for e in range(NUM_EXPERTS):
        toks, slots = np.nonzero(idx == e)
        tok_lists.append((toks, slots))
    cmax = max(len(t) for t, _ in tok_lists)

    n_chunks = max(1, -(-cmax // 512))
    CH = -(-cmax // (n_chunks * 16)) * 16  # chunk width, multiple of 16
    C_pad = CH * n_chunks

    in_maps = []
    for e in range(NUM_EXPERTS):
        toks, _ = tok_lists[e]
        x = np.zeros((C_pad, D), np.float32)
        x[: len(toks)] = hs[toks]
        xt = x.T.reshape(KT1, P, C_pad).transpose(1, 0, 2).astype(np_bf16)
        # [p, mt, kt, m] with the mt axis pair-interleaved: proj tile mt and
        # gate tile mt+MT1H land adjacently so pair mt needs one DMA chunk.
        w1e = w1[e].reshape(KT1, P, MT1, P).transpose(1, 2, 0, 3)[:, _W1_ORDER]
        w1e = w1e.astype(np_bf16)
        w2e = w2[e].reshape(KT2, P, MT2, P).transpose(1, 2, 0, 3).astype(np_bf16)
        in_maps.append({"xt": xt, "w1": w1e, "w2": w2e})

    nc = _get_graph(C_pad, n_chunks, CH)
    res = run_bass_kernel_spmd(
        nc,
        in_maps,
        core_ids=list(range(NCORES)),
        trace=trace,
        **(trace_kwargs or {}),
    )

    out = np.zeros((N, D), np.float32)
    for e in range(NUM_EXPERTS):
        toks, slots = tok_lists[e]
        oT = np.asarray(res.results[e]["out"], np.float32).reshape(HIDDEN, C_pad)
        out[toks] += sc[toks, slots][:, None] * oT[:, : len(toks)].T
    return out, res


def kernel(hidden_states, router_logits, w1, w2):
    out, _ = run(hidden_states, router_logits, w1, w2)
    return out
